# revision 31
# baseline (speedup 1.0000x reference)
"""Distributed Trainium2 kernel for nn_ActionEmbeddingModel.

Reference computation (B=4096, DC=1024, A=20000, C=128, H=1024):
    h         = relu(context @ w1 + b1)          # [B, H]
    ctx_score = h @ w2[:H]                       # [B]
    act_score = emb @ w2[H:]                     # [A]
    out[b, a] = ctx_score[b] + act_score[a] + b2 # [B, A]

Sharding (8 cores): pure data-parallel over the batch; emb and weights are
replicated so every core computes all act scores locally - NO collectives
(a cross-core barrier + AllGather measured ~60 us of latency/skew).

The whole device data path runs in bf16 (rel-err gate 2e-2, measured
~3e-3): the [512, 20000] per-core output shard is written as bf16
(20.5 MB vs 41 MB f32) and up-cast on the host; inputs are bf16 reads
(5.2 MB). Total ~28.8 MB HBM traffic/core = ~72 us at the measured
~400 GB/s per-core DMA rate; everything else hides behind it.
(fp8 e4m3 fc1 was tried: rel err 2.5e-2 > gate. bf16 it is.)

Structure per core:
  - Inputs stream on the GpSimd SWDGE ring (emb c0-c1, ctx, w1, emb
    c2-19), tiny tables on the ACT ring, output tiles on the SP HWDGE
    ring - so no engine pays trigger costs on its critical path and
    reads/writes interleave at SDMA packet granularity.
  - act_score is matvec'd with a column-REPLICATED w2c stationary
    [128(C) x 128], so each [128, 512] matmul lands in PSUM already
    partition-broadcast. Chunks are 1024 wide (2 psum banks, double
    buffered); one CAST per chunk (DVE/ACT alternating) converts PSUM
    to a bf16 act_bc tile.
  - fc1 is a single [128, 512]-moving pass (64 matmuls; one LDWEIGHTS
    per matmul is the PE cost driver, so widest legal moving operand).
    ctx_score needs no transpose: h tiles [h, b] are the STATIONARY
    operand with w2h [128, 1] moving -> [128(b), 1] lands in PSUM.
  - out tiles [128, 4096] assembled from 4 chunk-adds each (DVE 3/4,
    ACT 1/4), DMA'd as 1 MB writes (20 per core).
"""

import numpy as np
import ml_dtypes

import concourse.bass as bass
import concourse.mybir as mybir
from concourse import bacc
from concourse import tile
from concourse.tile import TileContext
from concourse.bass_utils import run_bass_kernel_spmd

# Problem shape (hardcoded per harness contract).
B, DC, A, C, H = 4096, 1024, 20000, 128, 1024
N_CORES = 8
B_SH = B // N_CORES        # 512 batch rows per core
P = 128                    # partitions
KT = DC // P               # 8 contraction tiles for fc1
HT = H // P                # 8 hidden tiles
BT = B_SH // P             # 4 batch blocks of 128 rows
# Action chunks: 1024-wide (2 psum banks; every matvec matmul is 512-wide
# = exactly one bank, so accumulation never crosses banks). Last chunk 544.
A_W = [1024] * 19 + [544]
A_S = [1024 * i for i in range(20)]
NC_A = len(A_W)
MM_N = 512
# Output tiles: 4 chunks each -> [128, 4096] 1 MB DMAs (last 3616 wide).
O_W = [4096, 4096, 4096, 4096, 3616]
O_S = [4096 * i for i in range(5)]
NT_A = len(O_W)
F32 = mybir.dt.float32
BF16 = mybir.dt.bfloat16
BF16_NP = ml_dtypes.bfloat16

_CACHED_NC = None


def _build():
    nc = bacc.Bacc(num_devices=N_CORES)

    ctx_pp = nc.declare_dram_parameter("ctx_pp", [P, KT, B_SH], BF16, isOutput=False)
    w1_pp = nc.declare_dram_parameter("w1_pp", [HT, P, KT, P], BF16, isOutput=False)
    b1c = nc.declare_dram_parameter("b1c", [P, HT], F32, isOutput=False)
    w2h = nc.declare_dram_parameter("w2h", [P, HT], BF16, isOutput=False)
    w2cb = nc.declare_dram_parameter("w2cb", [C, P], BF16, isOutput=False)
    b2c = nc.declare_dram_parameter("b2c", [P, 1], F32, isOutput=False)
    embT = nc.declare_dram_parameter("embT", [C, A], BF16, isOutput=False)
    out_ext = nc.declare_dram_parameter("out", [B_SH, A], BF16, isOutput=True)

    relu = mybir.ActivationFunctionType.Relu
    ident = mybir.ActivationFunctionType.Identity

    with TileContext(nc, num_cores=N_CORES) as tc:
        with (
            tc.tile_pool(name="persist", bufs=1) as persist,
            tc.tile_pool(name="hts", bufs=9) as hp,
            tc.tile_pool(name="outp", bufs=8) as outp,
            tc.tile_pool(name="psum_f", bufs=2, space="PSUM") as ppf,
            tc.tile_pool(name="psum_v", bufs=2, space="PSUM") as ppv,
            tc.tile_pool(name="psum_c", bufs=2, space="PSUM") as ppc,
        ):
            # ---- tiny tables on the ACT HWDGE ring (keeps SWDGE streaming)
            w2cb_sb = persist.tile([C, P], BF16, tag="w2cb")
            nc.scalar.dma_start(out=w2cb_sb[:, :], in_=w2cb[:, :])
            b2_sb = persist.tile([P, 1], F32, tag="b2c")
            nc.scalar.dma_start(out=b2_sb[:, :], in_=b2c[:, :])
            b1_sb = persist.tile([P, HT], F32, tag="b1")
            nc.scalar.dma_start(out=b1_sb[:, :], in_=b1c[:, :])
            w2h_sb = persist.tile([P, HT], BF16, tag="w2h")
            nc.scalar.dma_start(out=w2h_sb[:, :], in_=w2h[:, :])

            # ---- ctx + w1 on the SP (sync) HWDGE ring: it starts draining
            # ---- ~3 us before the SWDGE ring and these gate fc1. The out
            # ---- DMAs share this ring but only start ~12 us after these
            # ---- 3.15 MB have drained - no head-of-line conflict.
            ctx_sb = persist.tile([P, KT * B_SH], BF16, tag="ctx")
            nc.sync.dma_start(
                out=ctx_sb[:, :].rearrange("p (kt n) -> p kt n", kt=KT),
                in_=ctx_pp[:, :, :],
            )
            w1_sbs = []
            w1_dmas = []
            for hb in range(HT):
                w = persist.tile([P, KT * P], BF16, tag=f"w1_{hb}")
                w1_dmas.append(nc.sync.dma_start(
                    out=w[:, :].rearrange("p (kt c) -> p kt c", kt=KT),
                    in_=w1_pp[hb, :, :, :],
                ))
                w1_sbs.append(w)
            G_W = [5120, 5120, 5120, 4640]   # emb DMA groups (5 chunks each)
            G_S = [5120 * g for g in range(4)]
            emb_gs = []
            w1_last_dma = None
            for g in range(4):
                e = persist.tile([C, G_W[g]], BF16, tag=f"embg{g}")
                d = nc.gpsimd.dma_start(
                    out=e[:, :], in_=embT[:, G_S[g]:G_S[g] + G_W[g]]
                )
                if g == 0:
                    # Serialize reads: emb (SWDGE ring) must not steal HBM
                    # bandwidth from ctx/w1 (sync ring), which gate fc1.
                    tile.add_dep_helper(
                        d.ins, w1_dmas[-1].ins, sync=False,
                        reason="DMA: emb after w1 (read priority)",
                    )
                emb_gs.append(e)

            def emb_slice(c):
                g = c // 5
                off = A_S[c] - G_S[g]
                return emb_gs[g][:, off:off + A_W[c]]

            ctx_col = persist.tile([P, BT], F32, tag="ctx_col")
            act_bcs = []
            for c in range(NC_A):
                abc = persist.tile([P, A_W[c]], BF16, tag=f"abc{c}")
                act_bcs.append(abc)

            def emit_act_chunk(c):
                """act chunk c: [128, A_W[c]] PSUM, already partition-
                broadcast via the replicated-w2c stationary; one CAST."""
                w = A_W[c]
                esl = emb_slice(c)
                ps = ppv.tile([P, w], F32, tag="mv_ps")
                for off in range(0, w, MM_N):
                    sw = min(MM_N, w - off)
                    nc.tensor.matmul(
                        ps[:, off:off + sw],
                        w2cb_sb[:, :],
                        esl[:, off:off + sw],
                        start=True,
                        stop=True,
                    )
                if c % 2 == 0:
                    nc.vector.tensor_copy(act_bcs[c][:, :], ps[:, :])
                else:
                    nc.scalar.copy(act_bcs[c][:, :], ps[:, :])

            PW = 2 * P  # fc1 pair width: 256 batch rows per pass

            # PE warm-up: ~6 us of dummy matmuls while ctx/w1 stream in, so
            # the HAM clock gate is at 2.4 GHz when fc1 starts (the PE sits
            # idle for the first ~12 us otherwise and runs fc1 at 1.2 GHz).
            warm_ps = ppf.tile([P, PW], F32, tag="h_ps")
            for _ in range(32):
                nc.tensor.matmul(
                    warm_ps[:, 0:P],
                    w2cb_sb[:, :],
                    w2cb_sb[:, 0:P],
                    start=True,
                    stop=True,
                )

            def emit_fc1_mms(pair, mv_chunks=()):
                """h tiles for batch rows pair*256..+256 ([128,256] moving).
                Act-chunk matvecs from mv_chunks are interleaved between
                h-blocks so act tiles keep flowing while fc1 owns the PE."""
                mv_chunks = list(mv_chunks)
                ht_tiles = []
                for ht in range(HT):
                    ps = ppf.tile([P, PW], F32, tag="h_ps")
                    for kt in range(KT):
                        nc.tensor.matmul(
                            ps[:, :],
                            w1_sbs[ht][:, kt * P:(kt + 1) * P],
                            ctx_sb[:, kt * B_SH + pair * PW:
                                   kt * B_SH + (pair + 1) * PW],
                            start=(kt == 0),
                            stop=(kt == KT - 1),
                        )
                    hts = hp.tile([P, PW], BF16, tag="ht")
                    nc.scalar.activation(
                        hts[:, :], ps[:, :], relu, bias=b1_sb[:, ht:ht + 1]
                    )
                    ht_tiles.append(hts)
                    if mv_chunks:
                        emit_act_chunk(mv_chunks.pop(0))
                return ht_tiles

            def emit_ctx_cols(pair, ht_tiles):
                """ctx_col for both 128-row halves of a pair (h tiles as
                STATIONARY, w2h moving -> [128(b),1] in PSUM). Emitted a
                few act chunks AFTER the pair's fc1 so the PE isn't stalled
                waiting on the pair's last relus (ACT-paced)."""
                for half in range(2):
                    bs = 2 * pair + half
                    pst = ppc.tile([P, 1], F32, tag="cs_ps")
                    for ht in range(HT):
                        nc.tensor.matmul(
                            pst[:, :],
                            ht_tiles[ht][:, half * P:(half + 1) * P],
                            w2h_sb[:, ht:ht + 1],
                            start=(ht == 0),
                            stop=(ht == HT - 1),
                        )
                    nc.scalar.add(ctx_col[:, bs:bs + 1], pst[:, :], b2_sb[:, 0:1])

            # PE stream order matches DMA arrival order: fc1 pair 0 (ctx+w1
            # land first), act chunks 0-1 while pair-0 relus drain on ACT,
            # ctx_col 0/1, more group-0 chunks, fc1 pair 1 with group-1/2
            # chunks woven between h-blocks, ctx_col 2/3, rest after.
            p0_tiles = emit_fc1_mms(0)
            emit_act_chunk(0)
            emit_act_chunk(1)
            emit_ctx_cols(0, p0_tiles)
            for c in range(2, 5):
                emit_act_chunk(c)
            p1_tiles = emit_fc1_mms(1, mv_chunks=range(5, 13))
            emit_ctx_cols(1, p1_tiles)
            for c in range(13, NC_A):
                emit_act_chunk(c)

            # ---- out tiles [128, O_W[t]] = 4 chunk-adds (act_bc[c] +
            # ---- ctx_col[:, bs]) on DVE (3/4) / ACT (1/4). The 1 MB out
            # ---- DMAs alternate between the SP and ACT HWDGE rings: one
            # ---- ring's trigger+sem-wait cycle (~3 us) can't sustain the
            # ---- 2.56 us/tile drain rate alone.
            # ---- Emission order = dependency-readiness order: chunk
            # ---- groups are PE-produced in t order and ctx_col 0/1 lands
            # ---- ~14 us before 2/3, so (bs01, low t) first, bs23 woven in
            # ---- once pair 1 is due. A bs-major order stalls the write
            # ---- stream ~10 us on late chunks.
            for bs in range(BT):
                for t in range(NT_A):
                    o_sb = outp.tile([P, O_W[t]], BF16, tag="osb")
                    for c in range(4 * t, min(4 * t + 4, NC_A)):
                        lo = A_S[c] - O_S[t]
                        dst = o_sb[:, lo:lo + A_W[c]]
                        if (bs * NC_A + c) % 4 == 3:
                            nc.scalar.activation(
                                dst, act_bcs[c][:, :], ident,
                                bias=ctx_col[:, bs:bs + 1],
                            )
                        else:
                            nc.vector.tensor_scalar_add(
                                dst, act_bcs[c][:, :], ctx_col[:, bs:bs + 1]
                            )
                    eng = nc.sync if (bs * NT_A + t) % 2 == 0 else nc.scalar
                    eng.dma_start(
                        out=out_ext[
                            bs * P:(bs + 1) * P, O_S[t]:O_S[t] + O_W[t]
                        ],
                        in_=o_sb[:, :],
                    )
    nc.finalize()
    return nc


def _get_nc():
    global _CACHED_NC
    if _CACHED_NC is None:
        _CACHED_NC = _build()
    return _CACHED_NC


def _in_maps(context, w1, b1, emb, w2, b2):
    context = np.asarray(context, dtype=np.float32)
    w1 = np.asarray(w1, dtype=np.float32)
    b1 = np.asarray(b1, dtype=np.float32)
    emb = np.asarray(emb, dtype=np.float32)
    w2 = np.asarray(w2, dtype=np.float32)
    b2 = np.asarray(b2, dtype=np.float32)

    # w1_pp[hb, p, kt, c] = w1[kt*P + p, hb*P + c]
    w1_pp = np.ascontiguousarray(
        w1.reshape(KT, P, HT, P).transpose(2, 1, 0, 3)
    ).astype(BF16_NP)
    b1c = np.ascontiguousarray(b1.reshape(HT, P).T)
    w2h = np.ascontiguousarray(w2[:H].reshape(HT, P).T).astype(BF16_NP)
    # w2cb[k, p] = w2[H + k] for every p: replicated stationary so the
    # act matvec output is partition-broadcast for free.
    w2cb = np.ascontiguousarray(
        np.broadcast_to(w2[H:].reshape(C, 1), (C, P))
    ).astype(BF16_NP)
    b2c = np.broadcast_to(b2.reshape(1, 1), (P, 1)).astype(np.float32).copy()
    embT = np.ascontiguousarray(emb.T).astype(BF16_NP)

    maps = []
    for i in range(N_CORES):
        ctx_sh = context[i * B_SH:(i + 1) * B_SH]
        # ctx_pp[p, kt, n] = context[n, kt*P + p]
        ctx_pp = np.ascontiguousarray(
            ctx_sh.T.reshape(KT, P, B_SH).transpose(1, 0, 2)
        ).astype(BF16_NP)
        maps.append({
            "ctx_pp": ctx_pp,
            "w1_pp": w1_pp,
            "b1c": b1c,
            "w2h": w2h,
            "w2cb": w2cb,
            "b2c": b2c,
            "embT": embT,
        })
    return maps


def kernel(context, w1, b1, emb, w2, b2, _trace=False, **_trace_kwargs):
    nc = _get_nc()
    maps = _in_maps(context, w1, b1, emb, w2, b2)
    res = run_bass_kernel_spmd(
        nc, maps, core_ids=list(range(N_CORES)), trace=_trace, **_trace_kwargs
    )
    out = np.empty((B, A), dtype=np.float32)
    for i in range(N_CORES):
        out[i * B_SH:(i + 1) * B_SH, :] = res.results[i]["out"].astype(np.float32)
    if _trace:
        return out, res
    return out
